# revision 23
# baseline (speedup 1.0000x reference)
"""Multi-head attention (B=2, S=2048, D=1024, H=16) on 8 Trainium2 cores.

Sharding: core c = (batch b, head-group hg) with b = c // 4, hg = c % 4.
Each core computes 4 heads of one batch element end-to-end and emits a
partial output projection; the host sums the 4 partials per batch and adds
(bv @ Wo + bo) (the value-bias term commutes through the softmax since the
attention weights sum to 1).

v2 layout strategy (vs the fp32r baseline):
  - Host pre-transposes q/k/v to x^T [D, S] and pre-marshals every tensor
    into its exact SBUF layout ([128, ...] partition-major) in bf16, so all
    DMAs are contiguous-row streams and the 384 PE transposes + PSUM->SBUF
    cast copies disappear entirely.
  - All matmuls run bf16 (full-rate on the PE; accumulation fp32 in PSUM).
  - Softmax denominator comes from a ones-column in V via the PV matmul;
    1/Z is computed with the fast custom-DVE reciprocal straight out of
    PSUM, broadcast across partitions on the otherwise-idle GpSimd engine,
    and applied with one DVE multiply. No PE broadcast, no 3.3us DVE
    reciprocals.
  - Emission order pipelines the whole program. Minimal pre-work (K rows
    for the first head-pair + Q0) precedes the first attention group; all
    other projections (V, remaining K/Q rows, later Q blocks) and the
    output projections are "filler" units pumped into the attention chunk
    stream by a cost-budgeted queue, keeping the PE dense so the HAM clock
    stays ramped (sparse PE streams measurably drop to ~1.2-1.6 GHz).
    Groups overlap: the previous group's trailing PV + softmax-normalize
    slot in behind the next group's first scores/exp chunk.
"""

import numpy as np
import ml_dtypes

import concourse.bacc as bacc
import concourse.mybir as mybir
import concourse.tile as tile
from concourse.bass_utils import run_bass_kernel_spmd

F32 = mybir.dt.float32
BF16 = mybir.dt.bfloat16
BF = ml_dtypes.bfloat16

S_FULL, D_FULL, NH_PER_CORE, DH = 2048, 1024, 4, 64
N_CORES = 8
B_FULL, H_FULL = 2, 16

P = 128
S, D, NH = S_FULL, D_FULL, NH_PER_CORE
NSL = NH * DH            # 256: projection slice width per core
KD = D // P              # 8 contraction tiles
NT = NSL // P            # 2 head-pairs
ST = S // P              # 16 s-tiles
SBLK = 512               # i-block width
NB = S // SBLK           # 4 i-blocks
JT = ST                  # 16 j-tiles
JC = 2                   # j-tiles per score/exp chunk
SH = S // 2              # half-sequence (DMA/pipeline granularity)


def build_core_program():
    nc = bacc.Bacc("TRN2", target_bir_lowering=False, debug=False)

    xq_d = [nc.dram_tensor(f"xq{h}", [P, KD, SBLK], BF16,
                           kind="ExternalInput") for h in range(4)]
    xk_d = [nc.dram_tensor(f"xk{h}", [P, KD, SBLK], BF16,
                           kind="ExternalInput") for h in range(4)]
    xv_d = [nc.dram_tensor(f"xv{h}", [P, KD, SBLK], BF16,
                           kind="ExternalInput") for h in range(4)]
    wq_d = nc.dram_tensor("wq", [P, KD, NSL], BF16, kind="ExternalInput")
    wk_d = nc.dram_tensor("wk", [P, KD, NSL], BF16, kind="ExternalInput")
    wv_d = nc.dram_tensor("wv", [P, KD, NSL], BF16, kind="ExternalInput")
    wo_d = nc.dram_tensor("wo", [P, NT, D], BF16, kind="ExternalInput")
    bq_d = nc.dram_tensor("bq", [P, NT], F32, kind="ExternalInput")
    bk_d = nc.dram_tensor("bk", [P, NT], F32, kind="ExternalInput")
    out_d = nc.dram_tensor("out", [S, D], F32, kind="ExternalOutput")

    with tile.TileContext(nc) as tc:
        with tc.tile_pool(name="persist", bufs=1) as pp, \
             tc.tile_pool(name="work", bufs=2) as pw, \
             tc.tile_pool(name="pa", bufs=1, space="PSUM") as pa, \
             tc.tile_pool(name="pb", bufs=1, space="PSUM") as psb:

            # ---- persistent SBUF tensors ----
            wq_sb = pp.tile([P, KD, NSL], BF16, name="wq")
            wk_sb = pp.tile([P, KD, NSL], BF16, name="wk")
            wv_sb = pp.tile([P, KD, NSL], BF16, name="wv")
            wo_sb = pp.tile([P, NT, D], BF16, name="wo")
            bq_sb = pp.tile([P, NT], F32, name="bq")
            bk_sb = pp.tile([P, NT], F32, name="bk")
            xq_sb = [pp.tile([P, KD, SBLK], BF16, name=f"xq{h}")
                     for h in range(4)]
            xk_sb = [pp.tile([P, KD, SBLK], BF16, name=f"xk{h}")
                     for h in range(4)]
            xv_sb = [pp.tile([P, KD, SBLK], BF16, name=f"xv{h}")
                     for h in range(4)]
            kT = [pp.tile([P, NT, SH], BF16, name=f"kT{h}") for h in range(2)]
            qT = [pp.tile([P, NT, SBLK], BF16, name=f"qT{b}") for b in range(NB)]
            # natural-layout V (+ ones column feeding the softmax denominator)
            v_sb = [pp.tile([P, JT // 2, NH, DH + 1], BF16, name=f"v{h}")
                    for h in range(2)]
            for h in range(2):
                nc.vector.memset(v_sb[h][:, :, :, DH:DH + 1], 1.0)
            o_b = [pp.tile([P, NT, SBLK], BF16, name=f"o{b}") for b in range(NB)]

            # ---- DMAs, in pipeline-priority order ----
            def dma_flat(dst, dram):
                nc.sync.dma_start(dst.rearrange("p a b -> p (a b)"),
                                  dram.ap().rearrange("p a b -> p (a b)"))

            nc.sync.dma_start(bk_sb, bk_d.ap())
            dma_flat(wk_sb, wk_d)
            dma_flat(xk_sb[0], xk_d[0])
            dma_flat(xk_sb[1], xk_d[1])
            dma_flat(xk_sb[2], xk_d[2])
            dma_flat(xk_sb[3], xk_d[3])
            nc.sync.dma_start(bq_sb, bq_d.ap())
            dma_flat(wq_sb, wq_d)
            dma_flat(xq_sb[0], xq_d[0])
            dma_flat(wv_sb, wv_d)
            dma_flat(xv_sb[0], xv_d[0])
            dma_flat(xv_sb[1], xv_d[1])
            dma_flat(xv_sb[2], xv_d[2])
            dma_flat(xv_sb[3], xv_d[3])
            dma_flat(wo_sb, wo_d)
            dma_flat(xq_sb[1], xq_d[1])
            dma_flat(xq_sb[2], xq_d[2])
            dma_flat(xq_sb[3], xq_d[3])

            # ---- projection emitters ----
            def proj_qk_nt(x_sb, w_sb, b_sb, dst_ap, blk, nt, lead=False,
                           half=None, state={}):
                # one [128, 512] tile of the Q or K projection, [nsl, s] out.
                # half=0/1 emits only that kd-half (smaller filler unit);
                # the psum tile is carried in `state` between the halves.
                xh = x_sb[blk]
                coff = 0
                if half in (None, 0):
                    if lead:
                        ps = psb.tile([P, SBLK], F32, tag="ps_o", bufs=3,
                                      name="ps_proj")
                    else:
                        ps = pa.tile([P, SBLK], F32, tag="pa")
                    state[(id(dst_ap.tensor), blk, nt)] = ps
                else:
                    ps = state.pop((id(dst_ap.tensor), blk, nt))
                kds = range(KD) if half is None else                     range(half * KD // 2, (half + 1) * KD // 2)
                for kd in kds:
                    nc.tensor.matmul(
                        ps,
                        lhsT=w_sb[:, kd, nt * P:(nt + 1) * P],
                        rhs=xh[:, kd, coff:coff + SBLK],
                        start=(kd == 0),
                        stop=(kd == KD - 1),
                    )
                if half in (None, 1):
                    nc.vector.tensor_scalar_add(dst_ap, ps, b_sb[:, nt:nt + 1])

            def proj_v(st):
                # one 128-row s-tile of the V projection, natural [s, nsl] out
                xh = xv_sb[st // 4]
                coff = (st % 4) * P
                ps = pa.tile([P, SBLK], F32, tag="pa")
                for kd in range(KD):
                    nc.tensor.matmul(
                        ps[:, 0:NSL],
                        lhsT=xh[:, kd, coff:coff + P],
                        rhs=wv_sb[:, kd, :],
                        start=(kd == 0),
                        stop=(kd == KD - 1),
                    )
                nc.vector.tensor_copy(
                    v_sb[st // 8][:, st % 8, :, 0:DH],
                    ps[:, 0:NSL].rearrange("p (h d) -> p h d", d=DH),
                )

            # ---- attention emitters ----
            def scores(ib, hp, jc, ps_s):
                for jj in range(JC):
                    jt = jc * JC + jj
                    kTh = kT[jt // 8]
                    jcol = (jt % 8) * P
                    for h01 in range(2):
                        base = h01 * DH
                        nc.tensor.matmul(
                            ps_s[h01][:, jj, :],
                            lhsT=kTh[base:base + DH, hp, jcol:jcol + P],
                            rhs=qT[ib][base:base + DH, hp, :],
                            start=True,
                            stop=True,
                            tile_position=(base, 0),
                        )

            def exp_chunk(ps_s, p_tiles):
                for h01 in range(2):
                    nc.scalar.activation(
                        p_tiles[h01], ps_s[h01],
                        mybir.ActivationFunctionType.Exp,
                        scale=float(1.0 / np.sqrt(DH)),
                    )

            def pv_chunk(hp, jc, p_tiles, ps_o):
                for h01 in range(2):
                    h = hp * 2 + h01
                    for jj in range(JC):
                        jt = jc * JC + jj
                        nc.tensor.matmul(
                            ps_o[h01][0:DH + 1, :],
                            lhsT=v_sb[jt // 8][:, jt % 8, h, :],
                            rhs=p_tiles[h01][:, jj, :],
                            start=(jt == 0),
                            stop=(jt == JT - 1),
                        )

            def norm(ib, hp, ps_o):
                # o = (exp-weighted V sums) / Z; Z sits in PSUM row DH.
                # Batched emission so the two h01 chains pipeline on DVE/Pool.
                zrs, recs, rbcs = [], [], []
                for h01 in range(2):
                    zr = pw.tile([1, SBLK], F32, tag="zrow", bufs=2)
                    nc.vector.tensor_copy(zr, ps_o[h01][DH:DH + 1, :])
                    zrs.append(zr)
                for h01 in range(2):
                    rec = pw.tile([1, SBLK], F32, tag="rec", bufs=2)
                    nc.vector.reciprocal_approx_fast(out=rec, in_=zrs[h01])
                    recs.append(rec)
                for h01 in range(2):
                    rbc = pw.tile([DH, SBLK], F32, tag="rbc", bufs=2)
                    nc.gpsimd.partition_broadcast(rbc, recs[h01])
                    rbcs.append(rbc)
                for h01 in range(2):
                    nc.vector.tensor_mul(
                        o_b[ib][h01 * DH:h01 * DH + DH, hp, :],
                        ps_o[h01][0:DH, :], rbcs[h01]
                    )

            def out_proj_st(ib, st, tail=False):
                # one 128-row output tile; each D half: matmuls -> copy -> DMA
                ob = pw.tile([P, D], F32, tag="ob", bufs=2)
                ss_off = (st % (SBLK // P)) * P
                for nb in range(D // SBLK):
                    if tail:
                        # attention is over: ping-pong on the idle ps_s ring
                        pso = psb.tile([P, JC, SBLK], F32, tag="ps_s", bufs=2,
                                       name="pso")[:, 0, :]
                    else:
                        pso = pa.tile([P, SBLK], F32, tag="pa", name="pso")
                    for t in range(NT):
                        nc.tensor.matmul(
                            pso,
                            lhsT=o_b[ib][:, t, ss_off:ss_off + P],
                            rhs=wo_sb[:, t, nb * SBLK:(nb + 1) * SBLK],
                            start=(t == 0),
                            stop=(t == NT - 1),
                        )
                    nc.vector.tensor_copy(ob[:, nb * SBLK:(nb + 1) * SBLK], pso)
                    nc.sync.dma_start(
                        out_d[st * P:(st + 1) * P, nb * SBLK:(nb + 1) * SBLK],
                        ob[:, nb * SBLK:(nb + 1) * SBLK])

            # ---- program order ----
            # PE filler queue of (cost_us, thunk): pumped by cost budget per
            # attention chunk so the PE stays dense without starving ACT.
            # Invariant: the two kd-halves of a projection are adjacent in
            # the queue (nothing else may allocate the single pa bank
            # between them).
            fillers = []
            debt = [0.0]

            def pump(budget):
                budget += debt[0]
                while fillers and budget > 0:
                    cost, thunk = fillers.pop(0)
                    thunk()
                    budget -= cost
                debt[0] = min(budget, 1.0)

            def qk_half(x_sb, w_sb, b_sb, dst_ap, blk, nt, half):
                proj_qk_nt(x_sb, w_sb, b_sb, dst_ap, blk, nt, half=half)

            # PE warmup: ~4us of dependency-free dummy matmuls so the HAM
            # clock is ramped when the first K projection lands (cold-start
            # matmuls otherwise run at ~0.65-1.2 GHz)
            warm = pw.tile([P, 64], BF16, tag="warm", bufs=1)
            nc.vector.memset(warm, 0.0)
            for _ in range(64):
                wps = psb.tile([P, SBLK], F32, tag="ps_o", bufs=3,
                               name="warmup")
                nc.tensor.matmul(wps[0:64, 0:64], lhsT=warm, rhs=warm,
                                 start=True, stop=True)

            # minimal pre-work before the first exp: K rows for hp0 (nt=0)
            # over all j, Q0 rows for hp0
            for blk in range(NB):
                proj_qk_nt(xk_sb, wk_sb, bk_sb,
                           kT[blk // 2][:, 0, (blk % 2) * SBLK:
                                        (blk % 2) * SBLK + SBLK],
                           blk, 0, lead=True)
            proj_qk_nt(xq_sb, wq_sb, bq_sb, qT[0][:, 0, :], 0, 0, lead=True)

            # deferred into group-0 fillers: K nt=1 rows, Q0 nt=1 rows
            for blk in range(NB):
                for half in range(2):
                    fillers.append(
                        (1.0, lambda b=blk, h=half: qk_half(
                            xk_sb, wk_sb, bk_sb,
                            kT[b // 2][:, 1, (b % 2) * SBLK:
                                       (b % 2) * SBLK + SBLK], b, 1, h)))
            for half in range(2):
                fillers.append(
                    (1.0, lambda h=half: qk_half(
                        xq_sb, wq_sb, bq_sb, qT[0][:, 1, :], 0, 1, h)))

            carry = [None]  # (hp_prev, pending, ib_prev, ps_o_prev)

            def flush_prev():
                if carry[0] is not None:
                    hp_p, pend_p, ib_p, ps_o_p = carry[0]
                    for item in pend_p:
                        pv_chunk(hp_p, *item, ps_o_p)
                    norm(ib_p, hp_p, ps_o_p)
                    carry[0] = None

            for ib in range(NB):
                for hp in range(NT):
                    gi = ib * NT + hp
                    if hp == 1 and ib < NB - 1:
                        for nt in range(NT):
                            for half in range(2):
                                fillers.append(
                                    (1.0, lambda b=ib + 1, n=nt, h=half:
                                     qk_half(xq_sb, wq_sb, bq_sb,
                                             qT[b][:, n, :], b, n, h)))
                    ps_o = [
                        psb.tile([P, SBLK], F32, tag="ps_o", bufs=3,
                                 name=f"ps_o{h01}")
                        for h01 in range(2)
                    ]
                    lag = 4 if gi == 0 else 1
                    pending = []
                    for jc in range(JT // JC):
                        ps_s = [
                            psb.tile([P, JC, SBLK], F32, tag="ps_s", bufs=2,
                                     name=f"ps_s{h01}")
                            for h01 in range(2)
                        ]
                        p_tiles = [
                            pw.tile([P, JC, SBLK], BF16, tag=f"p{h01}",
                                    bufs=5, name="p_sb")
                            for h01 in range(2)
                        ]
                        scores(ib, hp, jc, ps_s)
                        exp_chunk(ps_s, p_tiles)
                        if jc == 0:
                            # previous group's trailing PV + norm slot in
                            # behind this group's first exp
                            flush_prev()
                        if gi == 0:
                            proj_v(2 * jc)
                            proj_v(2 * jc + 1)
                        pending.append((jc, p_tiles))
                        if len(pending) > lag:
                            pv_chunk(hp, *pending.pop(0), ps_o)
                        if gi == 0:
                            pump(0.5)
                        elif jc >= 2:
                            pump(1.05)
                    if gi == 0:
                        # K nt=1 / Q0 nt=1 must complete before group (0,1):
                        # drain the group-0 pendings and lead fillers now
                        for item in pending:
                            pv_chunk(hp, *item, ps_o)
                        norm(ib, hp, ps_o)
                        pump(1e9)
                    else:
                        carry[0] = (hp, pending, ib, ps_o)
                    if hp == 1 and ib < NB - 1:
                        fillers += [
                            (1.2, lambda b=ib, s=st: out_proj_st(b, s))
                            for st in range(ib * (SBLK // P),
                                            (ib + 1) * (SBLK // P))
                        ]
            flush_prev()
            # drain remaining fillers, then the last i-block's output
            # projection on the now-idle ps_s ring
            pump(1e9)
            for st in range((NB - 1) * (SBLK // P), NB * (SBLK // P)):
                out_proj_st(NB - 1, st, tail=True)

    nc.finalize()
    return nc



_NC_CACHE = {}


def _get_program():
    if "nc" not in _NC_CACHE:
        _NC_CACHE["nc"] = build_core_program()
    return _NC_CACHE["nc"]


def _marshal_xt(x):
    # [S, D] fp32 -> four [P, KD, 512] bf16 quarters of x^T in SBUF layout
    xt = np.ascontiguousarray(x.T).astype(BF)          # [D, S]
    xt = xt.reshape(KD, P, S).transpose(1, 0, 2)       # [P, KD, S]
    return tuple(np.ascontiguousarray(xt[:, :, q * SBLK:(q + 1) * SBLK])
                 for q in range(4))


def make_in_maps(q, k, v, Wq, bq, Wk, bk, Wv, bv, Wo, bo):
    q, k, v = (np.asarray(x, np.float32) for x in (q, k, v))
    Wq, Wk, Wv, Wo = (np.asarray(x, np.float32) for x in (Wq, Wk, Wv, Wo))
    bq, bk = np.asarray(bq, np.float32), np.asarray(bk, np.float32)
    B = q.shape[0]
    GROUPS = N_CORES // B

    xqs = [_marshal_xt(q[b]) for b in range(B)]
    xks = [_marshal_xt(k[b]) for b in range(B)]
    xvs = [_marshal_xt(v[b]) for b in range(B)]

    in_maps = []
    for c in range(N_CORES):
        b, hg = c // GROUPS, c % GROUPS
        sl = slice(hg * NSL, (hg + 1) * NSL)

        def wslice(W):
            ws = W[:, sl].astype(BF)                      # [D, NSL]
            return np.ascontiguousarray(
                ws.reshape(KD, P, NSL).transpose(1, 0, 2))

        wo_sl = Wo[sl, :].astype(BF)                      # [NSL, D]
        wo_m = np.ascontiguousarray(
            wo_sl.reshape(NT, P, D).transpose(1, 0, 2))

        m = {f"xq{q}": xqs[b][q] for q in range(4)}
        m.update({f"xk{q}": xks[b][q] for q in range(4)})
        m.update({f"xv{q}": xvs[b][q] for q in range(4)})
        in_maps.append({
            **m,
            "wq": wslice(Wq), "wk": wslice(Wk), "wv": wslice(Wv),
            "wo": wo_m,
            "bq": np.ascontiguousarray(bq[sl].reshape(NT, P).T),
            "bk": np.ascontiguousarray(bk[sl].reshape(NT, P).T),
        })
    return in_maps


def kernel(q, k, v, Wq, bq, Wk, bk, Wv, bv, Wo, bo):
    bv = np.asarray(bv, np.float32)
    bo = np.asarray(bo, np.float32)
    Wo_f = np.asarray(Wo, np.float32)
    B = np.asarray(q).shape[0]
    GROUPS = N_CORES // B

    nc = _get_program()
    in_maps = make_in_maps(q, k, v, Wq, bq, Wk, bk, Wv, bv, Wo, bo)
    res = run_bass_kernel_spmd(nc, in_maps, list(range(N_CORES)))

    out = np.zeros((B, S, D), np.float32)
    for c in range(N_CORES):
        out[c // GROUPS] += res.results[c]["out"]
    # bv commutes through the softmax (weights sum to 1): fold bv@Wo + bo here
    out += (bv @ Wo_f + bo)[None, None, :]
    return out
